# revision 15
# baseline (speedup 1.0000x reference)
"""GQA attention kernel for 8 trn2 cores.

Sharding: core c -> (batch c//2, head-half c%2). Each core computes a partial
out-projection for its 8 KV heads / 4 query groups on one batch; host sums the
two half partials per batch and adds bo.

Device-side layout (per core), fp16 storage everywhere:
  x^T   [128, 8, 2048]  e-major chunks (no bias row; biases fused on DVE)
  Q^T   [128, 4, 2048]  group g duplicated on both 64-row halves (dup via DMA)
  K^T   [128, 4, 2048]  pgroup g = heads (2g, 2g+1) on row halves
  Vones [128, 16, 8, 65] V natural + ones column (row-sum trick)
  P     [128, 8, 2, 512] half-tiles per (g,qt): exp scores for (kb, head)
  aoT   [128, 4, 2048]  unnormalized until per-qt bulk normalize

Per (g,qt) block: score matmuls are K=64 row-tile pairs at tile_position
(0,0)/(64,0) streaming concurrently into one [128,2,512] psum pair; exp of
each kb tile runs on ScalarE (exact, softmax scale folded into the
activation) or on DVE via a Schraudolph int16 bit-trick writing an fp16 bit
pattern, split to balance engine load. PV accumulates per head into a
[65,512] psum whose row 64 is the softmax denominator (ones column).
Normalization: denominators are collected into a [8, qt, 512] tile,
bulk-reciprocal'd per qt, broadcast on GpSimd and multiplied into aoT in
place. Out-projection + DMA per token block follows the last group's
attention for that qt.
"""

import numpy as np

import concourse.bass as bass
import concourse.tile as tile
from concourse import bacc, mybir
from concourse.bass_utils import run_bass_kernel_spmd

B, S, E = 4, 2048, 1024
NH, NG, HD = 16, 8, 64
HPG = NH // NG            # heads per group = 2
SCALE = HD ** -0.5
NCORES = 8
EC = 8                    # e-chunks (contraction 1024)
QT = 4                    # 512-wide q tiles
SB = 16                   # 128-row token blocks
KB = 16                   # 128-row k blocks

F16 = mybir.dt.float16
F32 = mybir.dt.float32
I16 = mybir.dt.int16

# Schraudolph exp constants for fp16 bit pattern: exp(s*SCALE) ~=
# bitcast_f16(int16(round(s*EXP_A + EXP_B))).
EXP_A = float(SCALE * np.log2(np.e) * 1024.0)
EXP_C = 60.0              # centering constant (tuned numerically)
EXP_B = float(15.0 * 1024.0 - EXP_C)
# kb indices whose exp runs on DVE via the bit trick (rest on ScalarE)
DVE_KBS = frozenset((3, 7, 11, 15))

_CACHE = {}
LAST_RESULT = None


def _build_program():
    from contextlib import ExitStack

    nc = bacc.Bacc("TRN2", target_bir_lowering=False, debug=False)
    x_d = nc.dram_tensor("x", [128, EC * S], F16, kind="ExternalInput").ap()
    wq_d = nc.dram_tensor("wq", [EC * 128, 256], F16, kind="ExternalInput").ap()
    wk_d = nc.dram_tensor("wk", [EC * 128, 512], F16, kind="ExternalInput").ap()
    wv_d = nc.dram_tensor("wv", [EC * 128, 512], F16, kind="ExternalInput").ap()
    wo_d = nc.dram_tensor("wo", [512, E], F16, kind="ExternalInput").ap()
    bq_d = nc.dram_tensor("bq", [128, 2], F32, kind="ExternalInput").ap()
    bk_d = nc.dram_tensor("bk", [128, 4], F32, kind="ExternalInput").ap()
    bv_d = nc.dram_tensor("bv", [1, 512], F32, kind="ExternalInput").ap()
    out_d = nc.dram_tensor("out", [S, E], F16, kind="ExternalOutput").ap()

    Exp = mybir.ActivationFunctionType.Exp
    Add = mybir.AluOpType.add
    Mult = mybir.AluOpType.mult

    with tile.TileContext(nc) as tc, ExitStack() as ctx:
        persist = ctx.enter_context(tc.tile_pool(name="persist", bufs=1))
        pt_pool = ctx.enter_context(tc.tile_pool(name="pt", bufs=3))
        rbc_pool = ctx.enter_context(tc.tile_pool(name="rbc", bufs=3))
        outp = ctx.enter_context(tc.tile_pool(name="outp", bufs=2))
        ps_sc = ctx.enter_context(tc.tile_pool(name="ps_sc", bufs=2, space="PSUM"))
        ps_pv = ctx.enter_context(tc.tile_pool(name="ps_pv", bufs=2, space="PSUM"))
        ps_lin = ctx.enter_context(tc.tile_pool(name="ps_lin", bufs=2, space="PSUM"))

        # ---- persistent SBUF tensors ----
        xT = persist.tile([128, EC, S], F16, tag="xT")
        wq = persist.tile([128, EC, 256], F16, tag="wq")
        wk = persist.tile([128, EC, 512], F16, tag="wk")
        wv = persist.tile([128, EC, 512], F16, tag="wv")
        wo = persist.tile([128, 4, E], F16, tag="wo")
        bqt = persist.tile([128, 2], F32, tag="bqt")
        bkt = persist.tile([128, 4], F32, tag="bkt")
        bvt = persist.tile([1, 512], F32, tag="bvt")
        bvb = persist.tile([128, 512], F32, tag="bvb")
        QTr = persist.tile([128, 4, S], F16, tag="QTr")
        KT = persist.tile([128, 4, S], F16, tag="KT")
        Vones = persist.tile([128, KB, 8, HD + 1], F16, tag="Vones")
        aoT = persist.tile([128, 4, S], F16, tag="aoT")

        # ---- loads ----
        xv = x_d.rearrange("p (c s) -> p c s", c=EC)
        wqv = wq_d.rearrange("(c p) n -> p c n", p=128)
        wkv = wk_d.rearrange("(c p) n -> p c n", p=128)
        # first block (g0, qt0) needs: wq gpair0, x quarter0, wk g0, biases
        nc.scalar.dma_start(out=wq[:, :, 0:128], in_=wqv[:, :, 0:128])
        nc.sync.dma_start(out=xT[:, :, 0:512], in_=xv[:, :, 0:512])
        nc.scalar.dma_start(out=bqt, in_=bq_d)
        nc.scalar.dma_start(out=wk[:, :, 0:128], in_=wkv[:, :, 0:128])
        nc.scalar.dma_start(out=bkt, in_=bk_d)
        nc.sync.dma_start(out=xT[:, :, 512:1024], in_=xv[:, :, 512:1024])
        nc.scalar.dma_start(out=wk[:, :, 128:256], in_=wkv[:, :, 128:256])
        nc.sync.dma_start(out=xT[:, :, 1024:1536], in_=xv[:, :, 1024:1536])
        nc.scalar.dma_start(out=xT[:, :, 1536:2048], in_=xv[:, :, 1536:2048])
        nc.sync.dma_start(out=wv, in_=wv_d.rearrange("(c p) n -> p c n", p=128))
        nc.scalar.dma_start(out=bvt, in_=bv_d)
        nc.sync.dma_start(out=wq[:, :, 128:256], in_=wqv[:, :, 128:256])
        nc.sync.dma_start(out=wk[:, :, 256:512], in_=wkv[:, :, 256:512])
        nc.scalar.dma_start(out=wo, in_=wo_d.rearrange("(c p) n -> p c n", p=128))
        nc.gpsimd.partition_broadcast(out_ap=bvb, in_ap=bvt)
        nc.vector.memset(Vones[:, :, :, HD:HD + 1], 1.0)

        # ---- projection emitters ----
        def proj_q(i):        # query group pair i -> groups (2i, 2i+1)
            for qt in range(QT):
                qs = slice(qt * 512, (qt + 1) * 512)
                ps = ps_lin.tile([128, 512], F32, tag="lin")
                for c in range(EC):
                    nc.tensor.matmul(
                        ps, lhsT=wq[:, c, i * 128:(i + 1) * 128],
                        rhs=xT[:, c, qs], start=(c == 0), stop=(c == EC - 1))
                nc.vector.tensor_scalar(
                    out=QTr[0:64, 2 * i, qs], in0=ps[0:64],
                    scalar1=bqt[0:64, i:i + 1], scalar2=None, op0=Add)
                nc.vector.tensor_scalar(
                    out=QTr[0:64, 2 * i + 1, qs], in0=ps[64:128],
                    scalar1=bqt[64:128, i:i + 1], scalar2=None, op0=Add)
            for g in (2 * i, 2 * i + 1):
                nc.sync.dma_start(out=QTr[64:128, g, :], in_=QTr[0:64, g, :])

        def proj_k(g):
            for qt in range(QT):
                qs = slice(qt * 512, (qt + 1) * 512)
                ps = ps_lin.tile([128, 512], F32, tag="lin")
                for c in range(EC):
                    nc.tensor.matmul(
                        ps, lhsT=wk[:, c, g * 128:(g + 1) * 128],
                        rhs=xT[:, c, qs], start=(c == 0), stop=(c == EC - 1))
                nc.vector.tensor_scalar(
                    out=KT[:, g, qs], in0=ps,
                    scalar1=bkt[:, g:g + 1], scalar2=None, op0=Add)

        def proj_v_range(lo, hi):
            for sb in range(lo, hi):
                proj_v(sb)

        def proj_v(sb):
            ps = ps_lin.tile([128, 512], F32, tag="lin")
            for c in range(EC):
                nc.tensor.matmul(
                    ps, lhsT=xT[:, c, sb * 128:(sb + 1) * 128],
                    rhs=wv[:, c, :], start=(c == 0), stop=(c == EC - 1))
            nc.vector.tensor_tensor(
                out=Vones[:, sb, :, 0:HD],
                in0=ps.rearrange("p (h d) -> p h d", h=8),
                in1=bvb.rearrange("p (h d) -> p h d", h=8), op=Add)

        # ---- attention emitters ----
        def scores_block(g, qt):
            """Emit scores+exp for one (g, qt); returns two pt half-tiles."""
            qs = slice(qt * 512, (qt + 1) * 512)
            pts = []
            for half in range(2):
                pt = pt_pool.tile([128, KB // 2, 2, 512], F16, tag="pt")
                pts.append(pt)
                for j in range(KB // 2):
                    kb = half * (KB // 2) + j
                    klo = slice(kb * 128, kb * 128 + 64)
                    khi = slice(kb * 128 + 64, (kb + 1) * 128)
                    ps = ps_sc.tile([128, 2, 512], F32, tag="sc")
                    nc.tensor.matmul(
                        ps[0:64, 0, :], lhsT=KT[0:64, g, klo],
                        rhs=QTr[0:64, g, qs],
                        start=True, stop=True, tile_position=(0, 0))
                    nc.tensor.matmul(
                        ps[64:128, 0, :], lhsT=KT[0:64, g, khi],
                        rhs=QTr[0:64, g, qs],
                        start=True, stop=True, tile_position=(0, 64))
                    nc.tensor.matmul(
                        ps[0:64, 1, :], lhsT=KT[64:128, g, klo],
                        rhs=QTr[64:128, g, qs],
                        start=True, stop=True, tile_position=(64, 0))
                    nc.tensor.matmul(
                        ps[64:128, 1, :], lhsT=KT[64:128, g, khi],
                        rhs=QTr[64:128, g, qs],
                        start=True, stop=True, tile_position=(64, 64))
                    if kb in DVE_KBS:
                        nc.vector.tensor_scalar(
                            out=pt[:, j, :, :].bitcast(I16), in0=ps,
                            scalar1=EXP_A, scalar2=EXP_B, op0=Mult, op1=Add)
                    else:
                        nc.scalar.activation(
                            out=pt[:, j, :, :], in_=ps, func=Exp,
                            scale=float(SCALE))
            return pts

        def pv_block(g, qt, pts):
            qs = slice(qt * 512, (qt + 1) * 512)
            for h in range(2):
                ps = ps_pv.tile([HD + 1, 512], F32, tag="pv")
                for kb in range(KB):
                    nc.tensor.matmul(
                        ps, lhsT=Vones[:, kb, 2 * g + h, :],
                        rhs=pts[kb // 8][:, kb % 8, h, :],
                        start=(kb == 0), stop=(kb == KB - 1))
                dn = rbc_pool.tile([1, 512], F32, tag="dn")
                nc.vector.tensor_copy(out=dn, in_=ps[HD:HD + 1, :])
                rc = rbc_pool.tile([1, 512], F32, tag="rc")
                # custom-DVE ops ignore the input partition offset; dn is at
                # partition 0 so reciprocal_approx_fast reads the right row
                nc.vector.reciprocal_approx_fast(out=rc, in_=dn)
                rb = rbc_pool.tile([64, 512], F32, tag="rb")
                nc.gpsimd.partition_broadcast(out_ap=rb, in_ap=rc)
                nc.vector.tensor_tensor(
                    out=aoT[h * 64:(h + 1) * 64, g, qs],
                    in0=ps[0:HD], in1=rb, op=Mult)

        def phase3_qt(qt):
            for sb in range(4 * qt, 4 * qt + 4):
                ss = slice(sb * 128, (sb + 1) * 128)
                for et in range(2):
                    es = slice(et * 512, (et + 1) * 512)
                    ps = ps_lin.tile([128, 512], F32, tag="lin")
                    for c in range(4):
                        nc.tensor.matmul(
                            ps, lhsT=aoT[:, c, ss], rhs=wo[:, c, es],
                            start=(c == 0), stop=(c == 3))
                    ot = outp.tile([128, 512], F16, tag="ot")
                    nc.vector.tensor_copy(out=ot, in_=ps)
                    nc.sync.dma_start(out=out_d[ss, es], in_=ot)

        # ---- emission: weave projections between early attention blocks
        # so the scalar engine never starves while the PE does projections
        proj_q(0)
        proj_k(0)
        prev = scores_block(0, 0) + [0, 0]   # [ptlo, pthi, g, qt]
        proj_k(1)

        blocks = [(g, qt) for qt in range(QT) for g in range(4)][1:]
        for g, qt in blocks:
            if (g, qt) == (2, 0):
                proj_q(1)
                proj_k(2)
            if (g, qt) == (3, 0):
                proj_k(3)
            pts = scores_block(g, qt)
            if (g, qt) == (1, 0):
                proj_v_range(0, 16)
            plo, phi, pg, pq = prev
            pv_block(pg, pq, (plo, phi))
            if pg == 3:
                phase3_qt(pq)
            prev = pts + [g, qt]
        plo, phi, pg, pq = prev
        pv_block(pg, pq, (plo, phi))
        phase3_qt(pq)

    nc.compile()
    return nc


def _prep_shards(x, Wq, bq, Wk, bk, Wv, bv, Wo):
    """Host-side shard prep. Returns per-core input maps (fp16 weights)."""
    f16 = np.float16
    # host-side transpose: x[b] [S, E] -> xT [128, EC, S] with e = c*128 + p
    xs = [np.ascontiguousarray(
        x[b].reshape(S, EC, 128).transpose(2, 1, 0)).astype(f16).reshape(128, EC * S)
        for b in range(B)]
    halves = []
    for half in range(2):
        wq_f = np.ascontiguousarray(Wq[:, half * 256:(half + 1) * 256])
        wk_f = np.ascontiguousarray(Wk[:, half * 512:(half + 1) * 512])
        wv_f = np.ascontiguousarray(Wv[:, half * 512:(half + 1) * 512])
        wo_f = np.ascontiguousarray(Wo[half * 512:(half + 1) * 512, :])
        bq_f = np.ascontiguousarray(
            bq[half * 256:(half + 1) * 256].reshape(2, 128).T)  # [128, 2]
        bk_f = np.ascontiguousarray(
            bk[half * 512:(half + 1) * 512].reshape(4, 128).T)  # [128, 4]
        bv_f = np.ascontiguousarray(
            bv[half * 512:(half + 1) * 512].reshape(1, 512))
        halves.append({
            "wq": wq_f.astype(f16), "wk": wk_f.astype(f16),
            "wv": wv_f.astype(f16), "wo": wo_f.astype(f16),
            "bq": bq_f.astype(np.float32), "bk": bk_f.astype(np.float32),
            "bv": bv_f.astype(np.float32),
        })
    in_maps = []
    for c in range(NCORES):
        m = {"x": xs[c // 2]}
        m.update(halves[c % 2])
        in_maps.append(m)
    return in_maps


def kernel(x, Wq, bq, Wk, bk, Wv, bv, Wo, bo):
    global LAST_RESULT
    x, Wq, bq, Wk, bk, Wv, bv, Wo, bo = [
        np.asarray(a, dtype=np.float32)
        for a in (x, Wq, bq, Wk, bk, Wv, bv, Wo, bo)]
    if "nc" not in _CACHE:
        _CACHE["nc"] = _build_program()
    nc = _CACHE["nc"]
    in_maps = _prep_shards(x, Wq, bq, Wk, bk, Wv, bv, Wo)
    res = run_bass_kernel_spmd(nc, in_maps, core_ids=list(range(NCORES)))
    LAST_RESULT = res
    out = np.empty((B, S, E), np.float32)
    for b in range(B):
        out[b] = (res.results[2 * b]["out"].astype(np.float32)
                  + res.results[2 * b + 1]["out"].astype(np.float32))
    out += bo.astype(np.float32)
    return out


# revision 16
# speedup vs baseline: 1.1670x; 1.1670x over previous
"""GQA attention kernel for 8 trn2 cores.

Sharding: core c -> (batch c//2, head-half c%2). Each core computes a partial
out-projection for its 8 KV heads / 4 query groups on one batch; host sums the
two half partials per batch and adds bo.

Device-side layout (per core), fp16 storage everywhere:
  x^T   [128, 8, 2048]  e-major chunks (no bias row; biases fused on DVE)
  Q^T   [128, 4, 2048]  group g duplicated on both 64-row halves (dup via DMA)
  K^T   [128, 4, 2048]  pgroup g = heads (2g, 2g+1) on row halves
  Vones [128, 16, 8, 65] V natural + ones column (row-sum trick)
  P     [128, 8, 2, 512] half-tiles per (g,qt): exp scores for (kb, head)
  aoT   [128, 4, 2048]  unnormalized until per-qt bulk normalize

Per (g,qt) block: score matmuls are K=64 row-tile pairs at tile_position
(0,0)/(64,0) streaming concurrently into one [128,2,512] psum pair; exp of
each kb tile runs on ScalarE (exact, softmax scale folded into the
activation) or on DVE via a Schraudolph int16 bit-trick writing an fp16 bit
pattern, split to balance engine load. PV accumulates per head into a
[65,512] psum whose row 64 is the softmax denominator (ones column).
Normalization: denominators are collected into a [8, qt, 512] tile,
bulk-reciprocal'd per qt, broadcast on GpSimd and multiplied into aoT in
place. Out-projection + DMA per token block follows the last group's
attention for that qt.
"""

import numpy as np

import concourse.bass as bass
import concourse.tile as tile
from concourse import bacc, mybir
from concourse.bass_utils import run_bass_kernel_spmd

B, S, E = 4, 2048, 1024
NH, NG, HD = 16, 8, 64
HPG = NH // NG            # heads per group = 2
SCALE = HD ** -0.5
NCORES = 8
EC = 8                    # e-chunks (contraction 1024)
QT = 4                    # 512-wide q tiles
SB = 16                   # 128-row token blocks
KB = 16                   # 128-row k blocks

F16 = mybir.dt.float16
F32 = mybir.dt.float32
I16 = mybir.dt.int16

# Schraudolph exp constants for fp16 bit pattern: exp(s*SCALE) ~=
# bitcast_f16(int16(round(s*EXP_A + EXP_B))).
EXP_A = float(SCALE * np.log2(np.e) * 1024.0)
EXP_C = 60.0              # centering constant (tuned numerically)
EXP_B = float(15.0 * 1024.0 - EXP_C)
# kb indices whose exp runs on DVE via the bit trick (rest on ScalarE)
DVE_KBS = frozenset((3, 7, 11, 15))

_CACHE = {}
LAST_RESULT = None


def _build_program():
    from contextlib import ExitStack

    nc = bacc.Bacc("TRN2", target_bir_lowering=False, debug=False)
    x_d = nc.dram_tensor("x", [128, EC * S], F16, kind="ExternalInput").ap()
    wq_d = nc.dram_tensor("wq", [EC * 128, 256], F16, kind="ExternalInput").ap()
    wk_d = nc.dram_tensor("wk", [EC * 128, 512], F16, kind="ExternalInput").ap()
    wv_d = nc.dram_tensor("wv", [EC * 128, 512], F16, kind="ExternalInput").ap()
    wo_d = nc.dram_tensor("wo", [512, E], F16, kind="ExternalInput").ap()
    bq_d = nc.dram_tensor("bq", [128, 2], F32, kind="ExternalInput").ap()
    bk_d = nc.dram_tensor("bk", [128, 4], F32, kind="ExternalInput").ap()
    bv_d = nc.dram_tensor("bv", [1, 512], F32, kind="ExternalInput").ap()
    out_d = nc.dram_tensor("out", [S, E], F16, kind="ExternalOutput").ap()

    Exp = mybir.ActivationFunctionType.Exp
    Add = mybir.AluOpType.add
    Mult = mybir.AluOpType.mult

    with tile.TileContext(nc) as tc, ExitStack() as ctx:
        persist = ctx.enter_context(tc.tile_pool(name="persist", bufs=1))
        pt_pool = ctx.enter_context(tc.tile_pool(name="pt", bufs=3))
        rbc_pool = ctx.enter_context(tc.tile_pool(name="rbc", bufs=3))
        outp = ctx.enter_context(tc.tile_pool(name="outp", bufs=2))
        ps_sc = ctx.enter_context(tc.tile_pool(name="ps_sc", bufs=2, space="PSUM"))
        ps_pv = ctx.enter_context(tc.tile_pool(name="ps_pv", bufs=2, space="PSUM"))
        ps_lin = ctx.enter_context(tc.tile_pool(name="ps_lin", bufs=2, space="PSUM"))

        # ---- persistent SBUF tensors ----
        xT = persist.tile([128, EC, S], F16, tag="xT")
        wq = persist.tile([128, EC, 256], F16, tag="wq")
        wk = persist.tile([128, EC, 512], F16, tag="wk")
        wv = persist.tile([128, EC, 512], F16, tag="wv")
        wo = persist.tile([128, 4, E], F16, tag="wo")
        bqt = persist.tile([128, 2], F32, tag="bqt")
        bkt = persist.tile([128, 4], F32, tag="bkt")
        bvt = persist.tile([1, 512], F32, tag="bvt")
        bvb = persist.tile([128, 512], F32, tag="bvb")
        QTr = persist.tile([128, 4, S], F16, tag="QTr")
        KT = persist.tile([128, 4, S], F16, tag="KT")
        Vones = persist.tile([128, KB, 8, HD + 1], F16, tag="Vones")
        aoT = persist.tile([128, 4, S], F16, tag="aoT")

        # ---- loads ----
        xv = x_d.rearrange("p (c s) -> p c s", c=EC)
        wqv = wq_d.rearrange("(c p) n -> p c n", p=128)
        wkv = wk_d.rearrange("(c p) n -> p c n", p=128)
        # first block (g0, qt0) needs: wq gpair0, x quarter0, wk g0, biases
        nc.scalar.dma_start(out=wq[:, :, 0:128], in_=wqv[:, :, 0:128])
        nc.sync.dma_start(out=xT[:, :, 0:512], in_=xv[:, :, 0:512])
        nc.scalar.dma_start(out=bqt, in_=bq_d)
        nc.scalar.dma_start(out=wk[:, :, 0:128], in_=wkv[:, :, 0:128])
        nc.scalar.dma_start(out=bkt, in_=bk_d)
        nc.sync.dma_start(out=xT[:, :, 512:1024], in_=xv[:, :, 512:1024])
        nc.scalar.dma_start(out=wk[:, :, 128:256], in_=wkv[:, :, 128:256])
        nc.sync.dma_start(out=xT[:, :, 1024:1536], in_=xv[:, :, 1024:1536])
        nc.scalar.dma_start(out=xT[:, :, 1536:2048], in_=xv[:, :, 1536:2048])
        nc.sync.dma_start(out=wv, in_=wv_d.rearrange("(c p) n -> p c n", p=128))
        nc.scalar.dma_start(out=bvt, in_=bv_d)
        nc.sync.dma_start(out=wq[:, :, 128:256], in_=wqv[:, :, 128:256])
        nc.sync.dma_start(out=wk[:, :, 256:512], in_=wkv[:, :, 256:512])
        nc.scalar.dma_start(out=wo, in_=wo_d.rearrange("(c p) n -> p c n", p=128))
        nc.gpsimd.partition_broadcast(out_ap=bvb, in_ap=bvt)
        nc.vector.memset(Vones[:, :, :, HD:HD + 1], 1.0)

        # ---- projection emitters ----
        def proj_q(i):        # query group pair i -> groups (2i, 2i+1)
            for qt in range(QT):
                qs = slice(qt * 512, (qt + 1) * 512)
                ps = ps_lin.tile([128, 512], F32, tag="lin")
                for c in range(EC):
                    nc.tensor.matmul(
                        ps, lhsT=wq[:, c, i * 128:(i + 1) * 128],
                        rhs=xT[:, c, qs], start=(c == 0), stop=(c == EC - 1))
                nc.vector.tensor_scalar(
                    out=QTr[0:64, 2 * i, qs], in0=ps[0:64],
                    scalar1=bqt[0:64, i:i + 1], scalar2=None, op0=Add)
                nc.vector.tensor_scalar(
                    out=QTr[0:64, 2 * i + 1, qs], in0=ps[64:128],
                    scalar1=bqt[64:128, i:i + 1], scalar2=None, op0=Add)
            for g in (2 * i, 2 * i + 1):
                nc.sync.dma_start(out=QTr[64:128, g, :], in_=QTr[0:64, g, :])

        def proj_k(g):
            for qt in range(QT):
                qs = slice(qt * 512, (qt + 1) * 512)
                ps = ps_lin.tile([128, 512], F32, tag="lin")
                for c in range(EC):
                    nc.tensor.matmul(
                        ps, lhsT=wk[:, c, g * 128:(g + 1) * 128],
                        rhs=xT[:, c, qs], start=(c == 0), stop=(c == EC - 1))
                nc.vector.tensor_scalar(
                    out=KT[:, g, qs], in0=ps,
                    scalar1=bkt[:, g:g + 1], scalar2=None, op0=Add)

        def proj_v_range(lo, hi):
            for sb in range(lo, hi):
                proj_v(sb)

        def proj_v(sb):
            ps = ps_lin.tile([128, 512], F32, tag="lin")
            for c in range(EC):
                nc.tensor.matmul(
                    ps, lhsT=xT[:, c, sb * 128:(sb + 1) * 128],
                    rhs=wv[:, c, :], start=(c == 0), stop=(c == EC - 1))
            nc.vector.tensor_tensor(
                out=Vones[:, sb, :, 0:HD],
                in0=ps.rearrange("p (h d) -> p h d", h=8),
                in1=bvb.rearrange("p (h d) -> p h d", h=8), op=Add)

        # ---- attention emitters ----
        def scores_block(g, qt):
            """Emit scores+exp for one (g, qt); returns two pt half-tiles."""
            qs = slice(qt * 512, (qt + 1) * 512)
            pts = []
            for half in range(2):
                pt = pt_pool.tile([128, KB // 2, 2, 512], F16, tag="pt")
                pts.append(pt)
                for j in range(KB // 2):
                    kb = half * (KB // 2) + j
                    ks = slice(kb * 128, (kb + 1) * 128)
                    ps = ps_sc.tile([128, 2, 512], F32, tag="sc")
                    nc.tensor.matmul(
                        ps[:, 0, :], lhsT=KT[0:64, g, ks], rhs=QTr[0:64, g, qs],
                        start=True, stop=True, tile_position=(0, 0))
                    nc.tensor.matmul(
                        ps[:, 1, :], lhsT=KT[64:128, g, ks],
                        rhs=QTr[64:128, g, qs],
                        start=True, stop=True, tile_position=(64, 0))
                    if kb in DVE_KBS:
                        nc.vector.tensor_scalar(
                            out=pt[:, j, :, :].bitcast(I16), in0=ps,
                            scalar1=EXP_A, scalar2=EXP_B, op0=Mult, op1=Add)
                    else:
                        nc.scalar.activation(
                            out=pt[:, j, :, :], in_=ps, func=Exp,
                            scale=float(SCALE))
            return pts

        def pv_block(g, qt, pts):
            qs = slice(qt * 512, (qt + 1) * 512)
            for h in range(2):
                ps = ps_pv.tile([HD + 1, 512], F32, tag="pv")
                for kb in range(KB):
                    nc.tensor.matmul(
                        ps, lhsT=Vones[:, kb, 2 * g + h, :],
                        rhs=pts[kb // 8][:, kb % 8, h, :],
                        start=(kb == 0), stop=(kb == KB - 1))
                dn = rbc_pool.tile([1, 512], F32, tag="dn")
                nc.vector.tensor_copy(out=dn, in_=ps[HD:HD + 1, :])
                rc = rbc_pool.tile([1, 512], F32, tag="rc")
                # custom-DVE ops ignore the input partition offset; dn is at
                # partition 0 so reciprocal_approx_fast reads the right row
                nc.vector.reciprocal_approx_fast(out=rc, in_=dn)
                rb = rbc_pool.tile([64, 512], F32, tag="rb")
                nc.gpsimd.partition_broadcast(out_ap=rb, in_ap=rc)
                nc.vector.tensor_tensor(
                    out=aoT[h * 64:(h + 1) * 64, g, qs],
                    in0=ps[0:HD], in1=rb, op=Mult)

        def phase3_qt(qt):
            for sb in range(4 * qt, 4 * qt + 4):
                ss = slice(sb * 128, (sb + 1) * 128)
                for et in range(2):
                    es = slice(et * 512, (et + 1) * 512)
                    ps = ps_lin.tile([128, 512], F32, tag="lin")
                    for c in range(4):
                        nc.tensor.matmul(
                            ps, lhsT=aoT[:, c, ss], rhs=wo[:, c, es],
                            start=(c == 0), stop=(c == 3))
                    ot = outp.tile([128, 512], F16, tag="ot")
                    nc.vector.tensor_copy(out=ot, in_=ps)
                    nc.sync.dma_start(out=out_d[ss, es], in_=ot)

        # ---- emission: weave projections between early attention blocks
        # so the scalar engine never starves while the PE does projections
        proj_q(0)
        proj_k(0)
        prev = scores_block(0, 0) + [0, 0]   # [ptlo, pthi, g, qt]
        proj_k(1)

        blocks = [(g, qt) for qt in range(QT) for g in range(4)][1:]
        for g, qt in blocks:
            if (g, qt) == (2, 0):
                proj_q(1)
                proj_k(2)
            if (g, qt) == (3, 0):
                proj_k(3)
            pts = scores_block(g, qt)
            if (g, qt) == (1, 0):
                proj_v_range(0, 16)
            plo, phi, pg, pq = prev
            pv_block(pg, pq, (plo, phi))
            if pg == 3:
                phase3_qt(pq)
            prev = pts + [g, qt]
        plo, phi, pg, pq = prev
        pv_block(pg, pq, (plo, phi))
        phase3_qt(pq)

    nc.compile()
    return nc


def _prep_shards(x, Wq, bq, Wk, bk, Wv, bv, Wo):
    """Host-side shard prep. Returns per-core input maps (fp16 weights)."""
    f16 = np.float16
    # host-side transpose: x[b] [S, E] -> xT [128, EC, S] with e = c*128 + p
    xs = [np.ascontiguousarray(
        x[b].reshape(S, EC, 128).transpose(2, 1, 0)).astype(f16).reshape(128, EC * S)
        for b in range(B)]
    halves = []
    for half in range(2):
        wq_f = np.ascontiguousarray(Wq[:, half * 256:(half + 1) * 256])
        wk_f = np.ascontiguousarray(Wk[:, half * 512:(half + 1) * 512])
        wv_f = np.ascontiguousarray(Wv[:, half * 512:(half + 1) * 512])
        wo_f = np.ascontiguousarray(Wo[half * 512:(half + 1) * 512, :])
        bq_f = np.ascontiguousarray(
            bq[half * 256:(half + 1) * 256].reshape(2, 128).T)  # [128, 2]
        bk_f = np.ascontiguousarray(
            bk[half * 512:(half + 1) * 512].reshape(4, 128).T)  # [128, 4]
        bv_f = np.ascontiguousarray(
            bv[half * 512:(half + 1) * 512].reshape(1, 512))
        halves.append({
            "wq": wq_f.astype(f16), "wk": wk_f.astype(f16),
            "wv": wv_f.astype(f16), "wo": wo_f.astype(f16),
            "bq": bq_f.astype(np.float32), "bk": bk_f.astype(np.float32),
            "bv": bv_f.astype(np.float32),
        })
    in_maps = []
    for c in range(NCORES):
        m = {"x": xs[c // 2]}
        m.update(halves[c % 2])
        in_maps.append(m)
    return in_maps


def kernel(x, Wq, bq, Wk, bk, Wv, bv, Wo, bo):
    global LAST_RESULT
    x, Wq, bq, Wk, bk, Wv, bv, Wo, bo = [
        np.asarray(a, dtype=np.float32)
        for a in (x, Wq, bq, Wk, bk, Wv, bv, Wo, bo)]
    if "nc" not in _CACHE:
        _CACHE["nc"] = _build_program()
    nc = _CACHE["nc"]
    in_maps = _prep_shards(x, Wq, bq, Wk, bk, Wv, bv, Wo)
    res = run_bass_kernel_spmd(nc, in_maps, core_ids=list(range(NCORES)))
    LAST_RESULT = res
    out = np.empty((B, S, E), np.float32)
    for b in range(B):
        out[b] = (res.results[2 * b]["out"].astype(np.float32)
                  + res.results[2 * b + 1]["out"].astype(np.float32))
    out += bo.astype(np.float32)
    return out
